# revision 10
# baseline (speedup 1.0000x reference)
"""Multi-head attention block (QKV proj -> softmax attention -> out proj) for
Trainium2, SPMD across 8 NeuronCores.

Sharding: batch (B=2) x head-groups (4 groups of 4 heads). Core c handles
batch c//4 and heads [4*(c%4), 4*(c%4)+4). Each core computes its partial
output contribution (context @ wo_slice.T); the host sums the 4 head-group
partials per batch (tensor-parallel row-sharded wo => the all-reduce is the
host-side gather).

All matmuls run in bf16 with fp32 PSUM accumulation, N=512 per matmul
instruction (one PSUM bank), arranged in long accumulation chains so weight
loads overlap row streaming.

Softmax runs in the standard [query, key] orientation: the scalar engine's
exp writes probs AND their row-sum via accum_out (free-axis accumulate), so
no tensor-engine ones-matmuls are needed for denominators. Probs are
normalized in-place on the DVE ([128,1] reciprocal broadcast) and
transposed to [key, query] tiles through the DMA x-bar for the PV matmul.

Per-core kernel layout (everything [partition=128, free]):
  xT   [2048, 2048] bf16   x[b].T             (feature k on partitions)
  wqT/wkT/wvT [2048, 512]  w[heads_slice].T   (k on partitions)
  woT  [512, 2048]  bf16   wo[:, slice].T     (local d on partitions)
  out  [2048, 2048] fp32   partial output for batch b
"""

import sys

if "/opt/trn_rl_repo" not in sys.path:
    sys.path.insert(0, "/opt/trn_rl_repo")

from contextlib import ExitStack

import ml_dtypes
import numpy as np

import concourse.bacc as bacc
import concourse.tile as tile
from concourse import mybir
from concourse.bass_utils import run_bass_kernel_spmd

BF16 = mybir.dt.bfloat16
F32 = mybir.dt.float32

B, S, DIM = 2, 2048, 2048
HEADS, HD = 16, 128
P = 128
N_CORES = 8
HGROUPS = 4  # head groups (second shard axis is batch)
HPC = HEADS // HGROUPS  # heads per core = 4
DL = HPC * HD  # local head dims per core = 512
SCALE = 1.0 / float(np.sqrt(HD))

NK = DIM // P  # 16 contraction tiles for the projections
NMT = S // P  # 16 query tiles of 128 tokens
NNT = S // P  # 16 kv tiles of 128 tokens

_PROGRAM_CACHE = {}


def _emit(nc, tc, xT, wqT, wkT, wvT, woT, maskf, out):
    with_mask = maskf is not None
    with ExitStack() as octx:
        planes = octx.enter_context(tc.tile_pool(name="planes", bufs=1))
        # q/k with head_dim on partitions, tokens on free
        q_sb = [planes.tile([P, S], BF16, tag=f"q{h}", name=f"q{h}") for h in range(HPC)]
        k_sb = [planes.tile([P, S], BF16, tag=f"k{h}", name=f"k{h}") for h in range(HPC)]
        ctx_sb = [planes.tile([P, S], BF16, tag=f"ctx{h}", name=f"ctx{h}")
                  for h in range(HPC)]
        vv_pool = octx.enter_context(tc.tile_pool(name="vv", bufs=1))
        vvs = [vv_pool.tile([P, NNT, P], BF16, tag=f"vv{h}", name=f"vv{h}")
               for h in range(HPC)]

        # Shared PSUM layout (8 banks): scores 2x[128,1024], ctx 2x[128,512],
        # outproj 2x[128,512]. The projection phase borrows the scores tag.
        ps_sc = octx.enter_context(tc.tile_pool(name="ps_sc", bufs=2, space="PSUM"))
        ps_ctx = octx.enter_context(tc.tile_pool(name="ps_ctx", bufs=2, space="PSUM"))
        ps_out = octx.enter_context(tc.tile_pool(name="ps_out", bufs=2, space="PSUM"))

        # ---------------- Phase 1: QKV projections ----------------
        # Per (name, head, kv-half): one psum tile [128,1024], chain over all
        # 16 k-tiles (x2 m-chunks of 512) -> 32 matmuls per chain; weight
        # loads overlap the row stream inside the accumulation chain.
        with ExitStack() as ctx:
            wpool = ctx.enter_context(tc.tile_pool(name="wqkv", bufs=1))
            w_sb = {}
            for name, src in (("q", wqT), ("k", wkT), ("v", wvT)):
                w_sb[name] = wpool.tile([P, NK, DL], BF16, tag=f"w{name}",
                                        name=f"w{name}")
            vT_sb = [wpool.tile([P, S], BF16, tag=f"vt{h}", name=f"vt{h}")
                     for h in range(HPC)]
            xpool = ctx.enter_context(tc.tile_pool(name="xt", bufs=1))
            xts = [xpool.tile([P, S], BF16, tag=f"x{kt}", name=f"x{kt}")
                   for kt in range(NK)]
            for kt in range(NK):
                nc.sync.dma_start(xts[kt][:], xT[kt * P : (kt + 1) * P, :])
                for name, src in (("q", wqT), ("k", wkT), ("v", wvT)):
                    nc.gpsimd.dma_start(
                        w_sb[name][:, kt, :], src[kt * P : (kt + 1) * P, :]
                    )

            for h in range(HPC):
                for name, dsts in (("q", q_sb), ("k", k_sb), ("v", vT_sb)):
                    for half in range(2):
                        ps = ps_sc.tile([P, 1024], F32, tag="ps_sc")
                        for kt in range(NK):
                            for mc in range(2):
                                m0 = half * 1024 + mc * 512
                                nc.tensor.matmul(
                                    ps[:, mc * 512 : (mc + 1) * 512],
                                    w_sb[name][:, kt, h * P : (h + 1) * P],
                                    xts[kt][:, m0 : m0 + 512],
                                    start=(kt == 0),
                                    stop=(kt == NK - 1),
                                )
                        nc.any.tensor_copy(
                            dsts[h][:, half * 1024 : (half + 1) * 1024], ps[:]
                        )
                # v to [kv, d] orientation via DMA x-bar transpose
                nc.sync.dma_start(vvs[h][:], vT_sb[h][:], transpose=True)

        # ------- Phase 2: attention (standard orientation) + out proj -------
        with ExitStack() as ctx:
            wopool = ctx.enter_context(tc.tile_pool(name="wo", bufs=1))
            wo_sb = [wopool.tile([P, DIM], BF16, tag=f"wo{h}", name=f"wo{h}")
                     for h in range(HPC)]
            for h in range(HPC):
                nc.gpsimd.dma_start(wo_sb[h][:], woT[h * P : (h + 1) * P, :])

            pbm_pool = ctx.enter_context(tc.tile_pool(name="pbm", bufs=8))
            pbt_pool = ctx.enter_context(tc.tile_pool(name="pbt", bufs=3))
            stats = ctx.enter_context(tc.tile_pool(name="stats", bufs=8))
            ob_pool = ctx.enter_context(tc.tile_pool(name="ob", bufs=2))
            if with_mask:
                mpool = ctx.enter_context(tc.tile_pool(name="mask", bufs=3))

            def scores_unit(h, mt, pbt_dst, mtl):
                """scores (4 mm) + exp/accum + den/recip + normalize +
                x-bar transpose into pbt_dst[:, :, mtl, :]."""
                qt = q_sb[h][:, mt * P : (mt + 1) * P]
                pbm = pbm_pool.tile([P, S], BF16, tag="pbm", name="pbm")
                accs = stats.tile([P, 2], F32, tag="accs", name="accs")
                if with_mask:
                    mts = mpool.tile([P, S], F32, tag="mt", name="mts")
                    nc.gpsimd.dma_start(mts[:], maskf[mt * P : (mt + 1) * P, :])
                for c in range(2):
                    ps = ps_sc.tile([P, 1024], F32, tag="ps_sc")
                    for sub in range(2):
                        k0 = c * 1024 + sub * 512
                        nc.tensor.matmul(
                            ps[:, sub * 512 : (sub + 1) * 512],
                            qt,
                            k_sb[h][:, k0 : k0 + 512],
                            start=True,
                            stop=True,
                        )
                    if with_mask:
                        nc.vector.tensor_add(
                            ps[:], ps[:], mts[:, c * 1024 : (c + 1) * 1024]
                        )
                    nc.scalar.activation(
                        pbm[:, c * 1024 : (c + 1) * 1024],
                        ps[:],
                        mybir.ActivationFunctionType.Exp,
                        scale=SCALE,
                        accum_out=accs[:, c : c + 1],
                    )
                den = stats.tile([P, 1], F32, tag="den", name="den")
                nc.vector.tensor_add(den[:], accs[:, 0:1], accs[:, 1:2])
                rec = stats.tile([P, 1], F32, tag="rec", name="rec")
                nc.vector.reciprocal(rec[:], den[:])
                nc.vector.tensor_scalar_mul(pbm[:], pbm[:], rec[:])
                nc.sync.dma_start(pbt_dst[:, :, mtl, :], pbm[:], transpose=True)

            def make_pv(h, jb, pbt):
                def pv():
                    ps = ps_ctx.tile([P, 512], F32, tag="ps_ctx")
                    for nt in range(NNT):
                        nc.tensor.matmul(
                            ps[:],
                            vvs[h][:, nt, :],
                            pbt[:, nt, :, :],
                            start=(nt == 0),
                            stop=(nt == NNT - 1),
                        )
                    nc.vector.tensor_copy(
                        ctx_sb[h][:, jb * 512 : (jb + 1) * 512], ps[:]
                    )
                return pv

            obs = {}

            def outproj_chain(tt, ec):
                """out[tt, ec*512:...] partial: chain over the 4 heads."""
                ps = ps_out.tile([P, 512], F32, tag="ps_out")
                for h in range(HPC):
                    nc.tensor.matmul(
                        ps[:],
                        ctx_sb[h][:, tt * P : (tt + 1) * P],
                        wo_sb[h][:, ec * 512 : (ec + 1) * 512],
                        start=(h == 0),
                        stop=(h == HPC - 1),
                    )
                if tt not in obs:
                    obs[tt] = ob_pool.tile([P, DIM], F32, tag="ob", name="ob")
                nc.vector.tensor_copy(
                    obs[tt][:, ec * 512 : (ec + 1) * 512], ps[:]
                )
                if ec == 3:
                    nc.gpsimd.dma_start(
                        out[tt * P : (tt + 1) * P, :], obs.pop(tt)[:]
                    )

            # PV lags its scores block by TWO h-blocks so the exp -> reduce ->
            # normalize -> x-bar-transpose pipeline tail (~10-15us) is covered
            # by queued PE work; out-proj chains of jb are consumed during the
            # back half of jb+1 (ctx(h3, jb) lands after (jb+1, h1)).
            pv_pending = []
            op_pending = []  # (tt, ec) chains of the previous jb
            for jb in range(4):
                unit_idx = 0
                for h in range(HPC):
                    pbt = pbt_pool.tile([P, NNT, 4, P], BF16, tag="pbt",
                                        name="pbt")
                    for mtl in range(4):
                        scores_unit(h, 4 * jb + mtl, pbt, mtl)
                        if unit_idx >= 8:
                            for _ in range(2):
                                if op_pending:
                                    outproj_chain(*op_pending.pop(0))
                        unit_idx += 1
                    pv_pending.append(make_pv(h, jb, pbt))
                    if len(pv_pending) > 2:
                        pv_pending.pop(0)()
                while op_pending:
                    outproj_chain(*op_pending.pop(0))
                op_pending = [(4 * jb + i, ec) for i in range(4) for ec in range(4)]
            while pv_pending:
                pv_pending.pop(0)()
            while op_pending:
                outproj_chain(*op_pending.pop(0))


def _build(with_mask: bool):
    nc = bacc.Bacc("TRN2")
    xT = nc.dram_tensor("xT", [DIM, S], BF16, kind="ExternalInput")
    wqT = nc.dram_tensor("wqT", [DIM, DL], BF16, kind="ExternalInput")
    wkT = nc.dram_tensor("wkT", [DIM, DL], BF16, kind="ExternalInput")
    wvT = nc.dram_tensor("wvT", [DIM, DL], BF16, kind="ExternalInput")
    woT = nc.dram_tensor("woT", [DL, DIM], BF16, kind="ExternalInput")
    maskf = (
        nc.dram_tensor("maskf", [S, S], F32, kind="ExternalInput")
        if with_mask
        else None
    )
    out = nc.dram_tensor("out", [S, DIM], F32, kind="ExternalOutput")
    with tile.TileContext(nc) as tc:
        _emit(nc, tc, xT, wqT, wkT, wvT, woT, maskf, out)
    nc.finalize()
    return nc


def _get_program(with_mask: bool):
    if with_mask not in _PROGRAM_CACHE:
        _PROGRAM_CACHE[with_mask] = _build(with_mask)
    return _PROGRAM_CACHE[with_mask]


def _prep_in_maps(x, mask, wq, wk, wv, wo, with_mask):
    bf = ml_dtypes.bfloat16
    f32 = np.float32
    xTs = [np.ascontiguousarray(x[b].T.astype(bf)) for b in range(B)]
    if with_mask:
        # mask indexed [query, key] in this orientation; exp applies
        # scale AFTER the mask add, so pre-divide
        maskf = np.ascontiguousarray(mask[0, 0].astype(f32) / SCALE)
    in_maps = []
    for c in range(N_CORES):
        b = c // HGROUPS
        g = c % HGROUPS
        sl = slice(g * DL, (g + 1) * DL)
        m = {
            "xT": xTs[b],
            "wqT": np.ascontiguousarray(wq[sl, :].T.astype(bf)),
            "wkT": np.ascontiguousarray(wk[sl, :].T.astype(bf)),
            "wvT": np.ascontiguousarray(wv[sl, :].T.astype(bf)),
            "woT": np.ascontiguousarray(wo[:, sl].T.astype(bf)),
        }
        if with_mask:
            m["maskf"] = maskf
        in_maps.append(m)
    return in_maps


def run_sharded(x, mask, wq, wk, wv, wo, trace=False, trace_kwargs=None):
    """Run the SPMD kernel; returns (full_output, BassKernelResults)."""
    with_mask = bool(np.any(np.asarray(mask)))
    nc = _get_program(with_mask)
    in_maps = _prep_in_maps(
        np.asarray(x), np.asarray(mask), np.asarray(wq), np.asarray(wk),
        np.asarray(wv), np.asarray(wo), with_mask,
    )
    kw = {}
    if trace:
        kw["trace"] = True
        if trace_kwargs:
            kw["trace_kwargs"] = trace_kwargs
    res = run_bass_kernel_spmd(nc, in_maps, list(range(N_CORES)), **kw)
    out = np.zeros((B, S, DIM), np.float32)
    for c in range(N_CORES):
        out[c // HGROUPS] += res.results[c]["out"]
    return out, res


def kernel(**inputs):
    out, _ = run_sharded(
        inputs["x"], inputs["mask"], inputs["wq"], inputs["wk"], inputs["wv"],
        inputs["wo"],
    )
    return out


# revision 14
# speedup vs baseline: 1.0456x; 1.0456x over previous
"""Multi-head attention block (QKV proj -> softmax attention -> out proj) for
Trainium2, SPMD across 8 NeuronCores.

Sharding: batch (B=2) x head-groups (4 groups of 4 heads). Core c handles
batch c//4 and heads [4*(c%4), 4*(c%4)+4). Each core computes its partial
output contribution (context @ wo_slice.T); the host sums the 4 head-group
partials per batch (tensor-parallel row-sharded wo => the all-reduce is the
host-side gather).

All matmuls run in bf16 with fp32 PSUM accumulation, N=512 per matmul
instruction (one PSUM bank), arranged in long accumulation chains so weight
loads overlap row streaming.

Softmax runs in the standard [query, key] orientation: the scalar engine's
exp writes probs AND their row-sum via accum_out (free-axis accumulate), so
no tensor-engine ones-matmuls are needed for denominators. Probs are
normalized in-place on the DVE ([128,1] reciprocal broadcast) and
transposed to [key, query] tiles through the DMA x-bar for the PV matmul.

Per-core kernel layout (everything [partition=128, free]):
  xT   [2048, 2048] bf16   x[b].T             (feature k on partitions)
  wqT/wkT/wvT [2048, 512]  w[heads_slice].T   (k on partitions)
  woT  [512, 2048]  bf16   wo[:, slice].T     (local d on partitions)
  out  [2048, 2048] fp32   partial output for batch b
"""

import sys

if "/opt/trn_rl_repo" not in sys.path:
    sys.path.insert(0, "/opt/trn_rl_repo")

from contextlib import ExitStack

import ml_dtypes
import numpy as np

import concourse.bacc as bacc
import concourse.tile as tile
from concourse import mybir
from concourse.bass_utils import run_bass_kernel_spmd

BF16 = mybir.dt.bfloat16
F32 = mybir.dt.float32

B, S, DIM = 2, 2048, 2048
HEADS, HD = 16, 128
P = 128
N_CORES = 8
HGROUPS = 4  # head groups (second shard axis is batch)
HPC = HEADS // HGROUPS  # heads per core = 4
DL = HPC * HD  # local head dims per core = 512
SCALE = 1.0 / float(np.sqrt(HD))

NK = DIM // P  # 16 contraction tiles for the projections
NMT = S // P  # 16 query tiles of 128 tokens
NNT = S // P  # 16 kv tiles of 128 tokens

_PROGRAM_CACHE = {}


def _emit(nc, tc, xT, wqT, wkT, wvT, woT, maskf, out):
    with_mask = maskf is not None
    with ExitStack() as octx:
        planes = octx.enter_context(tc.tile_pool(name="planes", bufs=1))
        # q/k with head_dim on partitions, tokens on free
        q_sb = [planes.tile([P, S], BF16, tag=f"q{h}", name=f"q{h}") for h in range(HPC)]
        k_sb = [planes.tile([P, S], BF16, tag=f"k{h}", name=f"k{h}") for h in range(HPC)]
        ctx_sb = [planes.tile([P, S], BF16, tag=f"ctx{h}", name=f"ctx{h}")
                  for h in range(HPC)]
        vv_pool = octx.enter_context(tc.tile_pool(name="vv", bufs=1))
        vvs = [vv_pool.tile([P, NNT, P], BF16, tag=f"vv{h}", name=f"vv{h}")
               for h in range(HPC)]

        # Shared PSUM layout (8 banks): scores 3x[128,1024] (deep rotation so
        # the PE can sprint ahead of the scalar exp stream), plus a shared
        # 2x[128,512] pool for the PV-context and out-proj chains.
        ps_sc = octx.enter_context(tc.tile_pool(name="ps_sc", bufs=3, space="PSUM"))
        ps_small = octx.enter_context(
            tc.tile_pool(name="ps_small", bufs=2, space="PSUM")
        )

        # ---------------- Phase 1: QKV projections ----------------
        # Per (name, head, kv-half): one psum tile [128,1024], chain over all
        # 16 k-tiles (x2 m-chunks of 512) -> 32 matmuls per chain; weight
        # loads overlap the row stream inside the accumulation chain.
        with ExitStack() as ctx:
            wpool = ctx.enter_context(tc.tile_pool(name="wqkv", bufs=1))
            w_sb = {}
            for name, src in (("q", wqT), ("k", wkT), ("v", wvT)):
                w_sb[name] = wpool.tile([P, NK, DL], BF16, tag=f"w{name}",
                                        name=f"w{name}")
            vT_sb = [wpool.tile([P, S], BF16, tag=f"vt{h}", name=f"vt{h}")
                     for h in range(HPC)]
            xpool = ctx.enter_context(tc.tile_pool(name="xt", bufs=1))
            xts = [xpool.tile([P, S], BF16, tag=f"x{kt}", name=f"x{kt}")
                   for kt in range(NK)]
            for kt in range(NK):
                nc.sync.dma_start(xts[kt][:], xT[kt * P : (kt + 1) * P, :])
                for name, src in (("q", wqT), ("k", wkT), ("v", wvT)):
                    nc.gpsimd.dma_start(
                        w_sb[name][:, kt, :], src[kt * P : (kt + 1) * P, :]
                    )

            for h in range(HPC):
                for name, dsts in (("q", q_sb), ("k", k_sb), ("v", vT_sb)):
                    for half in range(2):
                        ps = ps_sc.tile([P, 1024], F32, tag="ps_sc")
                        for kt in range(NK):
                            for mc in range(2):
                                m0 = half * 1024 + mc * 512
                                nc.tensor.matmul(
                                    ps[:, mc * 512 : (mc + 1) * 512],
                                    w_sb[name][:, kt, h * P : (h + 1) * P],
                                    xts[kt][:, m0 : m0 + 512],
                                    start=(kt == 0),
                                    stop=(kt == NK - 1),
                                )
                        nc.any.tensor_copy(
                            dsts[h][:, half * 1024 : (half + 1) * 1024], ps[:]
                        )
                # v to [kv, d] orientation via DMA x-bar transpose
                nc.sync.dma_start(vvs[h][:], vT_sb[h][:], transpose=True)

        # ------- Phase 2: attention (standard orientation) + out proj -------
        with ExitStack() as ctx:
            wopool = ctx.enter_context(tc.tile_pool(name="wo", bufs=1))
            wo_sb = [wopool.tile([P, DIM], BF16, tag=f"wo{h}", name=f"wo{h}")
                     for h in range(HPC)]
            for h in range(HPC):
                nc.gpsimd.dma_start(wo_sb[h][:], woT[h * P : (h + 1) * P, :])

            pbm_pool = ctx.enter_context(tc.tile_pool(name="pbm", bufs=8))
            pbt_pool = ctx.enter_context(tc.tile_pool(name="pbt", bufs=3))
            stats = ctx.enter_context(tc.tile_pool(name="stats", bufs=8))
            ob_pool = ctx.enter_context(tc.tile_pool(name="ob", bufs=2))
            if with_mask:
                mpool = ctx.enter_context(tc.tile_pool(name="mask", bufs=3))

            def scores_unit(h, mt, pbt_dst, mtl):
                """scores (4 mm) + exp/accum + den/recip + normalize +
                x-bar transpose into pbt_dst[:, :, mtl, :]."""
                qt = q_sb[h][:, mt * P : (mt + 1) * P]
                pbm = pbm_pool.tile([P, S], BF16, tag="pbm", name="pbm")
                accs = stats.tile([P, 2], F32, tag="accs", name="accs")
                if with_mask:
                    mts = mpool.tile([P, S], F32, tag="mt", name="mts")
                    nc.gpsimd.dma_start(mts[:], maskf[mt * P : (mt + 1) * P, :])
                for c in range(2):
                    ps = ps_sc.tile([P, 1024], F32, tag="ps_sc")
                    for sub in range(2):
                        k0 = c * 1024 + sub * 512
                        nc.tensor.matmul(
                            ps[:, sub * 512 : (sub + 1) * 512],
                            qt,
                            k_sb[h][:, k0 : k0 + 512],
                            start=True,
                            stop=True,
                        )
                    if with_mask:
                        nc.vector.tensor_add(
                            ps[:], ps[:], mts[:, c * 1024 : (c + 1) * 1024]
                        )
                    nc.scalar.activation(
                        pbm[:, c * 1024 : (c + 1) * 1024],
                        ps[:],
                        mybir.ActivationFunctionType.Exp,
                        scale=SCALE,
                        accum_out=accs[:, c : c + 1],
                    )
                den = stats.tile([P, 1], F32, tag="den", name="den")
                nc.vector.tensor_add(den[:], accs[:, 0:1], accs[:, 1:2])
                rec = stats.tile([P, 1], F32, tag="rec", name="rec")
                nc.vector.reciprocal(rec[:], den[:])
                nc.vector.tensor_scalar_mul(pbm[:], pbm[:], rec[:])
                nc.sync.dma_start(pbt_dst[:, :, mtl, :], pbm[:], transpose=True)

            def make_pv(h, jb, pbt):
                def pv():
                    ps = ps_small.tile([P, 512], F32, tag="ps_small")
                    for nt in range(NNT):
                        nc.tensor.matmul(
                            ps[:],
                            vvs[h][:, nt, :],
                            pbt[:, nt, :, :],
                            start=(nt == 0),
                            stop=(nt == NNT - 1),
                        )
                    nc.vector.tensor_copy(
                        ctx_sb[h][:, jb * 512 : (jb + 1) * 512], ps[:]
                    )
                return pv

            obs = {}

            def outproj_chain(tt, ec):
                """out[tt, ec*512:...] partial: chain over the 4 heads."""
                ps = ps_small.tile([P, 512], F32, tag="ps_small")
                for h in range(HPC):
                    nc.tensor.matmul(
                        ps[:],
                        ctx_sb[h][:, tt * P : (tt + 1) * P],
                        wo_sb[h][:, ec * 512 : (ec + 1) * 512],
                        start=(h == 0),
                        stop=(h == HPC - 1),
                    )
                if tt not in obs:
                    obs[tt] = ob_pool.tile([P, DIM], F32, tag="ob", name="ob")
                nc.vector.tensor_copy(
                    obs[tt][:, ec * 512 : (ec + 1) * 512], ps[:]
                )
                if ec == 3:
                    nc.gpsimd.dma_start(
                        out[tt * P : (tt + 1) * P, :], obs.pop(tt)[:]
                    )

            # PV lags its scores block by TWO h-blocks so the exp -> reduce ->
            # normalize -> x-bar-transpose pipeline tail (~10-15us) is covered
            # by queued PE work; out-proj chains of jb are consumed during the
            # back half of jb+1 (ctx(h3, jb) lands after (jb+1, h1)).
            # out-proj chains of jb become consumable after PV(h3, jb), which
            # (with the 2-block PV lag) lands at the end of (jb+1, h1). Takes
            # are paced across units 8..15 of jb+1 AND units 0..3 of jb+2 so
            # the PE has independent work across the jb boundary, where the
            # scores stream stalls on the scalar exp backlog.
            TAKES = {8: 2, 9: 1, 10: 1, 11: 1, 12: 1, 13: 1, 14: 1, 15: 2,
                     0: 2, 1: 2, 2: 1, 3: 1}
            pv_pending = []
            op_pending = []  # ready (tt, ec) chains (PV(h3) of their jb done)
            op_next = []  # chains whose ctx is not complete yet
            for jb in range(4):
                for h in range(HPC):
                    pbt = pbt_pool.tile([P, NNT, 4, P], BF16, tag="pbt",
                                        name="pbt")
                    for mtl in range(4):
                        unit_idx = 4 * h + mtl
                        scores_unit(h, 4 * jb + mtl, pbt, mtl)
                        for _ in range(TAKES.get(unit_idx, 0)):
                            if op_pending:
                                outproj_chain(*op_pending.pop(0))
                    pv_pending.append(make_pv(h, jb, pbt))
                    if len(pv_pending) > 2:
                        pv_pending.pop(0)()
                    if h == 1 and op_next:
                        # PV(h3, jb-1) just ran; its jb's chains are ready
                        op_pending = op_next + op_pending
                        op_next = []
                op_next = [(4 * jb + i, ec) for i in range(4) for ec in range(4)]
            while pv_pending:
                pv_pending.pop(0)()
            for chains in (op_pending, op_next):
                while chains:
                    outproj_chain(*chains.pop(0))


def _build(with_mask: bool):
    nc = bacc.Bacc("TRN2")
    xT = nc.dram_tensor("xT", [DIM, S], BF16, kind="ExternalInput")
    wqT = nc.dram_tensor("wqT", [DIM, DL], BF16, kind="ExternalInput")
    wkT = nc.dram_tensor("wkT", [DIM, DL], BF16, kind="ExternalInput")
    wvT = nc.dram_tensor("wvT", [DIM, DL], BF16, kind="ExternalInput")
    woT = nc.dram_tensor("woT", [DL, DIM], BF16, kind="ExternalInput")
    maskf = (
        nc.dram_tensor("maskf", [S, S], F32, kind="ExternalInput")
        if with_mask
        else None
    )
    out = nc.dram_tensor("out", [S, DIM], F32, kind="ExternalOutput")
    with tile.TileContext(nc) as tc:
        _emit(nc, tc, xT, wqT, wkT, wvT, woT, maskf, out)
    nc.finalize()
    return nc


def _get_program(with_mask: bool):
    if with_mask not in _PROGRAM_CACHE:
        _PROGRAM_CACHE[with_mask] = _build(with_mask)
    return _PROGRAM_CACHE[with_mask]


def _prep_in_maps(x, mask, wq, wk, wv, wo, with_mask):
    bf = ml_dtypes.bfloat16
    f32 = np.float32
    xTs = [np.ascontiguousarray(x[b].T.astype(bf)) for b in range(B)]
    if with_mask:
        # mask indexed [query, key] in this orientation; exp applies
        # scale AFTER the mask add, so pre-divide
        maskf = np.ascontiguousarray(mask[0, 0].astype(f32) / SCALE)
    in_maps = []
    for c in range(N_CORES):
        b = c // HGROUPS
        g = c % HGROUPS
        sl = slice(g * DL, (g + 1) * DL)
        m = {
            "xT": xTs[b],
            "wqT": np.ascontiguousarray(wq[sl, :].T.astype(bf)),
            "wkT": np.ascontiguousarray(wk[sl, :].T.astype(bf)),
            "wvT": np.ascontiguousarray(wv[sl, :].T.astype(bf)),
            "woT": np.ascontiguousarray(wo[:, sl].T.astype(bf)),
        }
        if with_mask:
            m["maskf"] = maskf
        in_maps.append(m)
    return in_maps


def run_sharded(x, mask, wq, wk, wv, wo, trace=False, trace_kwargs=None):
    """Run the SPMD kernel; returns (full_output, BassKernelResults)."""
    with_mask = bool(np.any(np.asarray(mask)))
    nc = _get_program(with_mask)
    in_maps = _prep_in_maps(
        np.asarray(x), np.asarray(mask), np.asarray(wq), np.asarray(wk),
        np.asarray(wv), np.asarray(wo), with_mask,
    )
    kw = {}
    if trace:
        kw["trace"] = True
        if trace_kwargs:
            kw["trace_kwargs"] = trace_kwargs
    res = run_bass_kernel_spmd(nc, in_maps, list(range(N_CORES)), **kw)
    out = np.zeros((B, S, DIM), np.float32)
    for c in range(N_CORES):
        out[c // HGROUPS] += res.results[c]["out"]
    return out, res


def kernel(**inputs):
    out, _ = run_sharded(
        inputs["x"], inputs["mask"], inputs["wq"], inputs["wk"], inputs["wv"],
        inputs["wo"],
    )
    return out
